# revision 4
# baseline (speedup 1.0000x reference)
"""Trainium2 Bass kernel for nn_ConstraintLayer (batched equality-constrained QP layer).

Math: the reference solves  M @ sol_i = [2*y_i; b_i]  for every batch row i,
with the SAME KKT matrix M = [[2I, A^T], [A, 0]] (80x80).  Since M is fixed,
    y_star = [2y, b] @ (M^{-1}[:64, :])^T  =  y @ Gy + b @ Gb
with Gy = 2*Minv[:64,:64].T (64x64) and Gb = Minv[:64,64:].T (16x64),
i.e. one skinny (batch,80)@(80,64) matmul — memory bound.

Distribution: pure data parallelism; the batch (1048576 rows) is split into 8
shards of 131072 rows, one per NeuronCore.  The tiny Gy/Gb factors are
precomputed once on host (float64 inverse of the 80x80 block matrix) and
replicated to every core.

Device layout (per core): the host pre-transposes each shard into
feature-major blocks so that every DMA is a contiguous [128-partition x 16KB]
2MB transfer (full SDMA engine coverage), and the TensorE consumes 512-column
moving tiles directly:

  * batch is processed in "chunks" of 512 rows; chunks are packed in PAIRS so
    each PSUM bank [128, 512] holds outputs for two chunks (even chunk ->
    partitions 0-63, odd chunk -> partitions 64-127).
  * Y blocks  [128, 4096]: partition = 64*parity + feature(64), col = 512*pairidx + row.
  * B blocks  [128, 4096]: partition = 32*a + 16*parity + feature(16), col = 512*colblk + row
    (a = pair % 4, colblk = (pair % 32) // 4).
  * Per pair only TWO matmuls: a K=128 matmul with the block-diagonal
    stationary Wy = [[Gy,0],[0,Gy]] computes BOTH chunks' y-contribution at
    once, then a K=32 matmul with a zero-interleaved Wb strip accumulates both
    chunks' b-contribution.
  * PSUM -> SBUF copy on VectorE, then contiguous 2MB DMA out (same layout as
    Y blocks); the host inverts the packing.
"""

import numpy as np

BATCH = 1048576
IN_DIM = 64
OUT_DIM = 16
N_CORES = 8
SHARD = BATCH // N_CORES        # 131072
CHUNK = 512                     # batch rows per matmul (one PSUM bank col-span)
PAIRS_PER_YBLK = 8              # Y/out block [128, 4096] = 8 pairs = 16 chunks
N_YBLK = SHARD // (2 * CHUNK * PAIRS_PER_YBLK)   # 16
N_BBLK = 4                      # B block [128, 4096] = 32 pairs
YCOLS = 512 * PAIRS_PER_YBLK    # 4096

_prog_cache = {}
last_results = None             # BassKernelResults of the most recent run (for test harness)


def _build_weights(A):
    """Host precompute of the stationary matrices (float64 inverse, cast f32)."""
    m, n = A.shape  # (16, 64)
    A64 = np.asarray(A, dtype=np.float64)
    M = np.zeros((n + m, n + m))
    M[:n, :n] = 2.0 * np.eye(n)
    M[:n, n:] = A64.T
    M[n:, :n] = A64
    Minv = np.linalg.inv(M)
    Gy = (2.0 * Minv[:n, :n].T).astype(np.float32)   # (64, 64)
    Gb = (Minv[:n, n:].T).astype(np.float32)         # (16, 64)

    Wy = np.zeros((128, 128), np.float32)
    Wy[:64, :64] = Gy
    Wy[64:, 64:] = Gy
    Wb_strip = np.zeros((32, 128), np.float32)
    Wb_strip[:16, :64] = Gb
    Wb_strip[16:, 64:] = Gb
    Wb = np.ascontiguousarray(np.tile(Wb_strip, (4, 1)))  # (128, 128)
    return Wy, Wb


def _pack_y(ys):
    # (131072, 64) -> blocks (16, 128, 4096); partition = 64*parity + f, col = 512*pairidx + s
    return np.ascontiguousarray(
        ys.reshape(N_YBLK, PAIRS_PER_YBLK, 2, CHUNK, 64).transpose(0, 2, 4, 1, 3)
    ).reshape(N_YBLK, 128, YCOLS)


def _pack_b(bs):
    # (131072, 16) -> blocks (4, 128, 4096); partition = 32a + 16*parity + i, col = 512*colblk + s
    return np.ascontiguousarray(
        bs.reshape(N_BBLK, 8, 4, 2, CHUNK, 16).transpose(0, 2, 3, 5, 1, 4)
    ).reshape(N_BBLK, 128, YCOLS)


def _unpack_out(ob):
    # inverse of _pack_y with feature dim 64: (16, 128, 4096) -> (131072, 64)
    return np.ascontiguousarray(
        ob.reshape(N_YBLK, 2, 64, PAIRS_PER_YBLK, CHUNK).transpose(0, 3, 1, 4, 2)
    ).reshape(SHARD, 64)


def _build_program():
    import concourse.bacc as bacc
    import concourse.mybir as mybir
    import concourse.tile as tile

    dt = mybir.dt.float32
    nc = bacc.Bacc("TRN2")
    Yt = nc.dram_tensor("Yt", (N_YBLK, 128, YCOLS), dt, kind="ExternalInput")
    Bt = nc.dram_tensor("Bt", (N_BBLK, 128, YCOLS), dt, kind="ExternalInput")
    Wy_d = nc.dram_tensor("Wy", (128, 128), dt, kind="ExternalInput")
    Wb_d = nc.dram_tensor("Wb", (128, 128), dt, kind="ExternalInput")
    Ot = nc.dram_tensor("Ot", (N_YBLK, 128, YCOLS), dt, kind="ExternalOutput")

    with tile.TileContext(nc) as tc:
        with (
            tc.tile_pool(name="wpool", bufs=1) as wpool,
            tc.tile_pool(name="ypool", bufs=3) as ypool,
            tc.tile_pool(name="bpool", bufs=2) as bpool,
            tc.tile_pool(name="opool", bufs=3) as opool,
            tc.tile_pool(name="pspool", bufs=4, space="PSUM") as pspool,
        ):
            wy = wpool.tile([128, 128], dt)
            nc.sync.dma_start(wy[:], Wy_d[:])
            wb = wpool.tile([128, 128], dt)
            nc.sync.dma_start(wb[:], Wb_d[:])

            btile = None
            for yb in range(N_YBLK):
                ytile = ypool.tile([128, YCOLS], dt)
                nc.sync.dma_start(ytile[:], Yt[yb])
                if yb % (N_YBLK // N_BBLK) == 0:
                    btile = bpool.tile([128, YCOLS], dt)
                    nc.sync.dma_start(btile[:], Bt[yb // (N_YBLK // N_BBLK)])
                otile = opool.tile([128, YCOLS], dt)
                for pi in range(PAIRS_PER_YBLK):
                    p = yb * PAIRS_PER_YBLK + pi   # global pair index 0..127
                    a = p % 4
                    cb = (p % 32) // 4
                    ps = pspool.tile([128, CHUNK], dt)
                    nc.tensor.matmul(
                        ps[:], wy[:], ytile[:, 512 * pi:512 * (pi + 1)],
                        start=True, stop=False,
                    )
                    nc.tensor.matmul(
                        ps[:], wb[32 * a:32 * a + 32, :],
                        btile[32 * a:32 * a + 32, 512 * cb:512 * (cb + 1)],
                        start=False, stop=True,
                        tile_position=(32 * a, 0),
                    )
                    nc.vector.tensor_copy(otile[:, 512 * pi:512 * (pi + 1)], ps[:])
                nc.sync.dma_start(Ot[yb], otile[:])
    nc.compile()  # bacc passes: split sync waits to HW limits, alloc regs, DCE
    return nc


def _get_program():
    if "nc" not in _prog_cache:
        _prog_cache["nc"] = _build_program()
    return _prog_cache["nc"]


def kernel(y, A, b):
    global last_results
    from concourse.bass_utils import run_bass_kernel_spmd

    y = np.ascontiguousarray(np.asarray(y, dtype=np.float32))
    b = np.ascontiguousarray(np.asarray(b, dtype=np.float32))
    A = np.asarray(A, dtype=np.float32)
    assert y.shape == (BATCH, IN_DIM) and b.shape == (BATCH, OUT_DIM)

    Wy, Wb = _build_weights(A)

    in_maps = []
    for core in range(N_CORES):
        ys = y[core * SHARD:(core + 1) * SHARD]
        bs = b[core * SHARD:(core + 1) * SHARD]
        in_maps.append({"Yt": _pack_y(ys), "Bt": _pack_b(bs), "Wy": Wy, "Wb": Wb})

    nc = _get_program()
    res = run_bass_kernel_spmd(nc, in_maps, core_ids=list(range(N_CORES)))
    last_results = res

    out = np.empty((BATCH, IN_DIM), np.float32)
    for core in range(N_CORES):
        out[core * SHARD:(core + 1) * SHARD] = _unpack_out(res.results[core]["Ot"])
    return out
